# revision 8
# baseline (speedup 1.0000x reference)
"""AdaptMarginSVLS loss kernel for 8 TRN2 NeuronCores (fp8 DoubleRow version).

Computes (loss, loss_ce, loss_margin) for
  inputs  [1, 16, 2048, 2048] f32
  targets [1, 2048, 2048] int64 (values 0..15)

loss_ce     = mean_pixels[ logsumexp_c(x) - x_t ]
loss_margin = mean_{c,h,w} | box3x3(onehot(t))/9 - x |   (zero-padded labels)
loss        = loss_ce + loss_margin

Sharding: H split into 8 x 256 rows. Each core receives its slab in a
row-PAIR layout (partition p holds rows 2p and 2p+1 as two free-dim
planes), with x pre-scaled by 9 and quantized to fp8-e4m3 on host.
The 3x3 label histogram is computed per class with fp8 DoubleRow
matmuls: one 256-deep tridiagonal contraction per 128-row output bank
covers the vertical 3-tap window including its halo rows, three
horizontally shifted moving views cover the horizontal taps, a 3-row
fixup matmul supplies the taps beyond the pair tile, and a -I DoubleRow
matmul subtracts 9x in the same PSUM accumulation. |psum| is
abs-accumulated on ACT/DVE, exp/logsumexp on ACT+DVE, and the picked
logit sum on GPSIMD. Each core emits per-partition partial sums
[margin_sum, lse_sum, picked9_sum]; the host reduces partitions/cores.
"""

import sys

sys.path.insert(0, "/opt/trn_rl_repo")

import numpy as np
import ml_dtypes

from contextlib import ExitStack

from concourse import bass, mybir, tile
from concourse.bass_utils import run_bass_kernel_spmd


def _split_excess_waits(nc):
    """This walrus build caps sync-wait commands per instruction
    ('Too many sync wait commands' in setupSyncWait). Tile can attach more.
    Split the excess semaphore waits onto same-engine nops inserted just
    before the offending instruction."""
    n_split = 0
    _MAX_WAITS = 1
    for fn in nc.m.functions:
        for bb in fn.blocks:
            out = []
            changed = False
            for inst in bb.instructions:
                si = getattr(inst, "sync_info", None)
                if si is not None and len(si.on_wait) > _MAX_WAITS:
                    waits = list(si.on_wait)
                    sem_w = [w for w in waits if w.sync_type == "semaphore"]
                    other = [w for w in waits if w.sync_type != "semaphore"]
                    budget = _MAX_WAITS - len(other)
                    assert budget >= 1, f"{inst.name}: non-sem waits {len(other)}"
                    keep, extra = sem_w[-budget:], sem_w[:-budget]
                    for k in range(0, len(extra), _MAX_WAITS):
                        n_split += 1
                        out.append(
                            mybir.InstNoOp(
                                name=f"{inst.name}-wsplit{k}",
                                engine=inst.engine,
                                bass_nofuse=True,
                                sync_info=mybir.SyncInfo(
                                    on_wait=extra[k : k + _MAX_WAITS], on_update=[]
                                ),
                            )
                        )
                    inst.sync_info = mybir.SyncInfo(
                        on_wait=other + keep, on_update=list(si.on_update)
                    )
                    changed = True
                out.append(inst)
            if changed:
                bb.instructions = out
    return n_split


NC = 16
H = 2048
W = 2048
HSH = H // 8          # 256 rows per core
BLK = 128             # partitions (row pairs)
WP = W + 2            # zero-padded label columns

BF16 = mybir.dt.bfloat16
FP8 = mybir.dt.float8e4
F32 = mybir.dt.float32
Alu = mybir.AluOpType
Act = mybir.ActivationFunctionType
DR = mybir.MatmulPerfMode.DoubleRow

W_CH = 512            # PSUM bank width (f32)
UNIT_W = 1024         # abs-accumulate granularity (2 banks)

FP8NP = ml_dtypes.float8_e4m3fn

# engine assignment knobs (tuned from trace)
ABS_ACT = set(range(16))          # classes whose |psum| pass runs on ACT
PSCR_GP = set()          # classes whose picked-sum stt runs on GPSIMD


def _stationaries(w=W):
    """fp8 DoubleRow stationaries, [p, e, m] with tile row i = 2p + e."""
    s0 = np.zeros((BLK, 2, BLK), dtype=np.float32)
    s1 = np.zeros((BLK, 2, BLK), dtype=np.float32)
    sx0 = np.zeros((BLK, 2, BLK), dtype=np.float32)
    sx1 = np.zeros((BLK, 2, BLK), dtype=np.float32)
    for p in range(BLK):
        for e in range(2):
            i = 2 * p + e
            for m in range(BLK):
                # bank A out row m <- t0 rows i = m+1..m+3 (global m-1..m+1)
                if 1 <= i - m <= 3:
                    s0[p, e, m] = 1.0
                # bank B out row 128+m <- t0 rows i = m+129..m+131
                if 129 <= i - m <= 131:
                    s1[p, e, m] = 1.0
                if i == m:
                    sx0[p, e, m] = -1.0
                if i == m + 128:
                    sx1[p, e, m] = -1.0
    # fixup: F tile rows i_F = 2q + e (global r0+252+i_F); out rows 125..127
    # of bank B miss taps i_F in {2,3,4} (global r0+254..r0+256)
    sf = np.zeros((BLK, 2, BLK), dtype=np.float32)
    for a in range(3):
        for q in range(3):
            for e in range(2):
                i_f = 2 * q + e
                for m in (125, 126, 127):
                    if 2 <= i_f <= 4 and m - 125 <= i_f <= m - 123:
                        sf[32 * a + q, e, m] = 1.0
    return (x.astype(FP8NP) for x in (s0, s1, sf, sx0, sx1))


def build_graph(w=W, split_waits=True):
    w_ch = min(W_CH, w)
    unit_w = min(UNIT_W, w)
    n_units = w // unit_w
    wp = w + 2
    nc = bass.Bass()
    x9p = nc.declare_dram_parameter("x9p", [NC, BLK, 2, w], FP8, isOutput=False)
    t0d = nc.declare_dram_parameter("t0", [BLK, 2, wp], BF16, isOutput=False)
    tcd = nc.declare_dram_parameter("tc", [BLK, 2, w], BF16, isOutput=False)
    tfd = nc.declare_dram_parameter("tF", [BLK, 2, wp], BF16, isOutput=False)
    out = nc.declare_dram_parameter("partials", [BLK, 8], F32, isOutput=True)

    s0n, s1n, sfn, sx0n, sx1n = _stationaries(w)
    s0d = nc.inline_tensor(s0n, name="s0")
    s1d = nc.inline_tensor(s1n, name="s1")
    sfd = nc.inline_tensor(sfn, name="sf")
    sx0d = nc.inline_tensor(sx0n, name="sx0")
    sx1d = nc.inline_tensor(sx1n, name="sx1")
    # per-partition class constants for the 6 fixup-onehot tiles:
    # tile g serves classes c = 3g + a at partitions 32a + q (q = 0..2);
    # PE base partitions are limited to {0, 32, 64}
    cpat_np = np.full((BLK, 6), -1.0, dtype=np.float32)
    for g in range(6):
        for a in range(3):
            if 3 * g + a < NC:
                for q in range(3):
                    cpat_np[32 * a + q, g] = float(3 * g + a)
    cpatd = nc.inline_tensor(cpat_np, name="cpatF")

    with tile.TileContext(nc) as tc, ExitStack() as ctx:
        cpool = ctx.enter_context(tc.tile_pool(name="const", bufs=1))
        xpool = ctx.enter_context(tc.tile_pool(name="x", bufs=1))
        opool = ctx.enter_context(tc.tile_pool(name="oh", bufs=2))
        wpool = ctx.enter_context(tc.tile_pool(name="wk", bufs=2))
        epool = ctx.enter_context(tc.tile_pool(name="ex", bufs=2))
        apool = ctx.enter_context(tc.tile_pool(name="acc", bufs=1))
        spool = ctx.enter_context(tc.tile_pool(name="small", bufs=2))
        ppool = ctx.enter_context(tc.tile_pool(name="ps", bufs=2, space="PSUM"))

        s0 = cpool.tile([BLK, 2, BLK], FP8, tag="s0")
        s1 = cpool.tile([BLK, 2, BLK], FP8, tag="s1")
        sf = cpool.tile([BLK, 2, BLK], FP8, tag="sf")
        sx0 = cpool.tile([BLK, 2, BLK], FP8, tag="sx0")
        sx1 = cpool.tile([BLK, 2, BLK], FP8, tag="sx1")
        cpat = cpool.tile([BLK, 6], F32, tag="cpat")
        for sb, dr in ((s0, s0d), (s1, s1d), (sf, sfd), (sx0, sx0d),
                       (sx1, sx1d), (cpat, cpatd)):
            nc.sync.dma_start(sb[:], dr[:])

        t0 = cpool.tile([BLK, 2, wp], BF16, tag="t0")
        tc_t = cpool.tile([BLK, 2, w], BF16, tag="tc")
        tf = cpool.tile([BLK, 2, wp], BF16, tag="tf")
        nc.sync.dma_start(t0[:], t0d[:])
        nc.sync.dma_start(tc_t[:], tcd[:])
        nc.sync.dma_start(tf[:], tfd[:])

        # all of x resident: [128, (c, e, w)] fp8
        xall = xpool.tile([BLK, NC, 2, w], FP8, tag="xall")
        for c in range(NC):
            nc.sync.dma_start(xall[:, c], x9p[c])

        # fixup onehots (4 class-groups, 32-aligned partition bases)
        f_oh = cpool.tile([BLK, 6, 2, wp], FP8, tag="f_oh")
        for g in range(6):
            nc.vector.tensor_scalar(
                f_oh[:, g], tf[:], cpat[:, g : g + 1], None, Alu.is_equal
            )

        # accumulators
        se0 = apool.tile([BLK, 2 * w], BF16, tag="se0")
        se1 = apool.tile([BLK, 2 * w], BF16, tag="se1")
        mcol = apool.tile([BLK, NC * 2 * n_units], F32, tag="mcol")
        pcol = apool.tile([BLK, NC], F32, tag="pcol")
        fin = apool.tile([BLK, 8], F32, tag="fin")
        nc.vector.memset(fin[:], 0.0)

        for c in range(NC):
            xc = xall[:, c]                       # [128, 2, w] fp8
            xc_f = xc.rearrange("p e w -> p (e w)")

            # CE: exp + running sum-of-exps
            ex = epool.tile([BLK, 2 * w], BF16, tag="ex")
            nc.scalar.activation(ex[:], xc_f, Act.Exp, scale=1.0 / 9.0)
            se_t = se0 if c % 2 == 0 else se1
            if c < 2:
                nc.vector.tensor_copy(se_t[:], ex[:])
            else:
                nc.vector.tensor_tensor(se_t[:], se_t[:], ex[:], Alu.add)

            # picked-logit partial: sum (t==c) * 9x
            pscr = wpool.tile([BLK, 2 * w], BF16, tag="pscr")
            eng = nc.gpsimd if c in PSCR_GP else nc.vector
            eng.scalar_tensor_tensor(
                pscr[:].rearrange("p (e w) -> p e w", e=2),
                tc_t[:], float(c), xc, Alu.is_equal, Alu.mult,
                accum_out=pcol[:, c : c + 1],
            )

            # onehot for the histogram matmuls
            oh = opool.tile([BLK, 2, wp], FP8, tag="oh")
            nc.vector.tensor_scalar(oh[:], t0[:], float(c), None, Alu.is_equal)

            g, a = divmod(c, 3)
            fo = f_oh[:, g]
            for u in range(n_units):
                u0 = u * unit_w
                psa = ppool.tile([BLK, unit_w], F32, tag="psa")
                psb = ppool.tile([BLK, unit_w], F32, tag="psb")
                n_j = unit_w // w_ch
                # bank A: out rows 0..127
                for dx in range(3):
                    for j in range(n_j):
                        s = j * w_ch
                        nc.tensor.matmul(
                            psa[:, s : s + w_ch], s0[:],
                            oh[:, :, u0 + s + dx : u0 + s + dx + w_ch],
                            start=(dx == 0), stop=False, perf_mode=DR,
                        )
                for j in range(n_j):
                    s = j * w_ch
                    nc.tensor.matmul(
                        psa[:, s : s + w_ch], sx0[:],
                        xc[:, :, u0 + s : u0 + s + w_ch],
                        start=False, stop=True, perf_mode=DR,
                    )
                # bank B: out rows 128..255
                for dx in range(3):
                    for j in range(n_j):
                        s = j * w_ch
                        nc.tensor.matmul(
                            psb[:, s : s + w_ch], s1[:],
                            oh[:, :, u0 + s + dx : u0 + s + dx + w_ch],
                            start=(dx == 0), stop=False, perf_mode=DR,
                        )
                for dx in range(3):
                    for j in range(n_j):
                        s = j * w_ch
                        nc.tensor.matmul(
                            psb[:, s : s + w_ch], sf[32 * a : 32 * a + 3],
                            fo[32 * a : 32 * a + 3, :,
                               u0 + s + dx : u0 + s + dx + w_ch],
                            start=False, stop=False, perf_mode=DR,
                        )
                for j in range(n_j):
                    s = j * w_ch
                    nc.tensor.matmul(
                        psb[:, s : s + w_ch], sx1[:],
                        xc[:, :, u0 + s : u0 + s + w_ch],
                        start=False, stop=True, perf_mode=DR,
                    )
                # margin: sum |psum|
                for k, ps in enumerate((psa, psb)):
                    idx = (c * n_units + u) * 2 + k
                    scr = spool.tile([BLK, unit_w], BF16, tag="scr")
                    if c in ABS_ACT:
                        nc.scalar.activation(
                            scr[:], ps[:], Act.Abs,
                            accum_out=mcol[:, idx : idx + 1],
                        )
                    else:
                        nc.vector.tensor_scalar(
                            scr[:], ps[:], 0.0, None, Alu.abs_max, Alu.add,
                            accum_out=mcol[:, idx : idx + 1],
                        )

        # epilogue
        nc.vector.tensor_tensor(se0[:], se0[:], se1[:], Alu.add)
        lscr = epool.tile([BLK, 2 * w], BF16, tag="lscr")
        nc.scalar.activation(lscr[:], se0[:], Act.Ln, accum_out=fin[:, 1:2])
        nc.vector.tensor_reduce(fin[:, 0:1], mcol[:], mybir.AxisListType.X, Alu.add)
        nc.vector.tensor_reduce(fin[:, 2:3], pcol[:], mybir.AxisListType.X, Alu.add)
        nc.sync.dma_start(out[:], fin[:])

    if split_waits:
        _split_excess_waits(nc)
    return nc


def shard_inputs(inputs, targets, w=W):
    """inputs [1,16,H,W] f32, targets [1,H,W] int -> per-core in_maps."""
    x = np.asarray(inputs)[0][:, :, :w]
    t = np.asarray(targets)[0][:, :w]
    wp = w + 2
    y8 = (x * np.float32(9.0)).astype(FP8NP)
    # padded label map: tg[i, j] = label(i - 2, j - 1), zeros outside
    tg = np.zeros((H + 4, wp), dtype=ml_dtypes.bfloat16)
    tg[2 : H + 2, 1 : w + 1] = t.astype(ml_dtypes.bfloat16)
    tb = t.astype(ml_dtypes.bfloat16)
    in_maps = []
    for i in range(8):
        r0 = i * HSH
        t0 = np.ascontiguousarray(tg[r0 : r0 + 256].reshape(BLK, 2, wp))
        tc = np.ascontiguousarray(tb[r0 : r0 + 256].reshape(BLK, 2, w))
        tfix = tg[r0 + 254 : r0 + 260].reshape(3, 2, wp)
        tf = np.zeros((BLK, 2, wp), dtype=ml_dtypes.bfloat16)
        for a in range(4):
            tf[32 * a : 32 * a + 3] = tfix
        x9 = np.ascontiguousarray(
            y8[:, r0 : r0 + 256].reshape(NC, BLK, 2, w)
        )
        in_maps.append({"x9p": x9, "t0": t0, "tc": tc, "tF": tf})
    return in_maps


def combine_partials(partials, w=W):
    """partials: list of 8 arrays [128, 8] f32 -> (loss, ce, margin) f32."""
    acc = np.zeros(8, dtype=np.float64)
    for p in partials:
        acc += np.asarray(p, dtype=np.float64).sum(axis=0)
    margin_sum, lse_sum, picked9_sum = acc[0], acc[1], acc[2]
    n_pix = float(H * w)
    margin = margin_sum / 9.0 / (NC * n_pix)
    ce = (lse_sum - picked9_sum / 9.0) / n_pix
    loss = ce + margin
    return (
        np.float32(loss),
        np.float32(ce),
        np.float32(margin),
    )


_CACHE = {}


def _run(inputs, targets, trace=False):
    if "nc" not in _CACHE:
        _CACHE["nc"] = build_graph()
    nc = _CACHE["nc"]
    in_maps = shard_inputs(inputs, targets)
    res = run_bass_kernel_spmd(nc, in_maps, core_ids=list(range(8)), trace=trace)
    partials = [r["partials"] for r in res.results]
    return combine_partials(partials), res


def kernel(inputs, targets):
    out, _ = _run(inputs, targets, trace=False)
    return out


if __name__ == "__main__":
    pass
